# revision 6
# baseline (speedup 1.0000x reference)
"""Trainium2 Bass kernel for multi-head attention (B=4, T=2048, C=1024, H=16).

Sharding: 8 cores = (batch b in 0..3) x (head-group g in 0..1, 8 heads each).
Per core: QKV projections for its 512 dims, attention for 8 heads, partial
output projection. Host sums the two per-batch partials and adds the biases
that fold out of the device computation:
  - bk drops entirely (softmax is invariant to per-query additive constants)
  - bv folds to host:   out += Wo @ bv   (softmax rows sum to 1)
  - bo added on host
  - bq is applied on-device on the Q-projection drain (DVE tensor_scalar);
    the 1/sqrt(dh) scale is folded into wq/bq on the host.

The kernel is paced by the scalar engine: softmax exp is 33.5M elements per
core at 1 elem/cycle/lane (~293us incl. per-instruction overhead), more than
all PE matmul work combined.  Schedule: K projection, Q projection for query
block 0, then attention group (qb0) starts immediately (priority-boosted) so
exp begins ~50us in; the V projection, remaining Q blocks, PV, normalize and
output projection all fill PE/DVE slack underneath the saturated ACT engine.

Device schedule (per core, emission order == program order):
  weights DMA; K proj (4 blocks); Q proj block 0; V proj (all, program-order
  before its PV consumers, de-prioritized below qb0 attention); for each
  query block: scores pairs (row-tiled, concurrent) -> exp on ACT ->
  ones-augmented P@V (row 64 = softmax denominator) -> reciprocal+broadcast
  normalize -> out-projection (bf16 out, summed on host).
"""
import numpy as np
import ml_dtypes

import concourse.bass as bass
import concourse.mybir as mybir
import concourse.tile as tile
from concourse import bacc

F32 = mybir.dt.float32
BF16 = mybir.dt.bfloat16
AF = mybir.ActivationFunctionType

B, T, C = 4, 2048, 1024
H, CH = 16, 64
G = 512            # dims per head-group (8 heads)
NCIN = 8           # 128-chunks of C
NCOUT = 4          # 128-chunks of G
NTB = 4            # 512-wide t blocks
NKC = 16           # 128-wide key chunks
NQB = 4            # 512-wide query blocks
SCALE = 1.0 / np.sqrt(CH)


def build_nc(debug=False):
    nc = bacc.Bacc()
    xq = nc.declare_dram_parameter("xq", [C, T], BF16, isOutput=False)
    xk = nc.declare_dram_parameter("xk", [C, T], BF16, isOutput=False)
    xv = nc.declare_dram_parameter("xv", [C, T], BF16, isOutput=False)
    wq = nc.declare_dram_parameter("wq", [C, G], BF16, isOutput=False)
    wk = nc.declare_dram_parameter("wk", [C, G], BF16, isOutput=False)
    wv = nc.declare_dram_parameter("wv", [C, G], BF16, isOutput=False)
    wo = nc.declare_dram_parameter("wo", [G, C], BF16, isOutput=False)
    bq = nc.declare_dram_parameter("bq", [128, NCOUT], F32, isOutput=False)
    out = nc.declare_dram_parameter("out", [T, C], BF16, isOutput=True)

    xq_r = xq.rearrange("(c p) t -> p c t", p=128)
    xk_r = xk.rearrange("(c p) t -> p c t", p=128)
    xv_r = xv.rearrange("(c p) t -> p c t", p=128)
    wk_r = wk.rearrange("(c p) g -> p c g", p=128)
    wq_r = wq.rearrange("(c p) g -> p c g", p=128)
    wv_r = wv.rearrange("(c p) g -> p c g", p=128)

    with tile.TileContext(nc) as tc:
        with tc.tile_pool(name="persist", bufs=1) as persist, \
             tc.tile_pool(name="xs", bufs=2) as xs, \
             tc.tile_pool(name="eb", bufs=2) as eb, \
             tc.tile_pool(name="otp", bufs=2) as otp, \
             tc.tile_pool(name="dv", bufs=1) as dv, \
             tc.tile_pool(name="pp", bufs=2, space="PSUM") as pp, \
             tc.tile_pool(name="scp", bufs=2, space="PSUM") as scp, \
             tc.tile_pool(name="pvp", bufs=2, space="PSUM") as pvp:
            qt = [persist.tile([128, T], BF16, tag=f"qt{i}", name=f"qt{i}")
                  for i in range(NCOUT)]
            kt = [persist.tile([128, T], BF16, tag=f"kt{i}", name=f"kt{i}")
                  for i in range(NCOUT)]
            # V augmented: per-head column 64 is ones -> PV row 64 = denominator
            v_aug = persist.tile([128, NKC, 8, 65], BF16, tag="vaug")
            nc.vector.memset(v_aug[:, :, :, 64:65], 1.0)

            wk_sb = persist.tile([128, NCIN, G], BF16, tag="wk")
            wq_sb = persist.tile([128, NCIN, G], BF16, tag="wq")
            wv_sb = persist.tile([128, NCIN, G], BF16, tag="wv")
            wo_sb = persist.tile([128, NCOUT, C], BF16, tag="wo")
            bq_sb = persist.tile([128, NCOUT], F32, tag="bq")
            for ci in range(NCIN):
                nc.default_dma_engine.dma_start(out=wk_sb[:, ci, :],
                                                in_=wk_r[:, ci, :])
            for ci in range(NCIN):
                nc.default_dma_engine.dma_start(out=wq_sb[:, ci, :],
                                                in_=wq_r[:, ci, :])
            nc.default_dma_engine.dma_start(out=bq_sb, in_=bq[:, :])

            def k_proj_block(tb, interleave_q=None):
                """K projection for one t-block; optionally interleave the
                matching Q projection per output chunk so scores for the
                first head pairs unblock as early as possible."""
                xk_t = xs.tile([128, NCIN, 512], BF16, tag="xs", name="xk_t")
                for ci in range(NCIN):
                    nc.default_dma_engine.dma_start(
                        out=xk_t[:, ci, :],
                        in_=xk_r[:, ci, tb * 512:(tb + 1) * 512])
                xq_t = None
                if interleave_q is not None:
                    xq_t = xs.tile([128, NCIN, 512], BF16, tag="xs",
                                   name="xq_t")
                    for ci in range(NCIN):
                        nc.default_dma_engine.dma_start(
                            out=xq_t[:, ci, :],
                            in_=xq_r[:, ci, tb * 512:(tb + 1) * 512])
                for co in range(NCOUT):
                    ps = pp.tile([128, 512], F32, tag="proj", name="psk")
                    for ci in range(NCIN):
                        nc.tensor.matmul(
                            ps, wk_sb[:, ci, co * 128:(co + 1) * 128],
                            xk_t[:, ci, :],
                            start=(ci == 0), stop=(ci == NCIN - 1))
                    nc.vector.tensor_copy(
                        out=kt[co][:, tb * 512:(tb + 1) * 512], in_=ps)
                    if interleave_q is not None:
                        psq = pp.tile([128, 512], F32, tag="proj", name="psq")
                        for ci in range(NCIN):
                            nc.tensor.matmul(
                                psq, wq_sb[:, ci, co * 128:(co + 1) * 128],
                                xq_t[:, ci, :],
                                start=(ci == 0), stop=(ci == NCIN - 1))
                        nc.vector.tensor_scalar_add(
                            qt[co][:, tb * 512:(tb + 1) * 512], psq,
                            bq_sb[:, co:co + 1])

            def q_proj_block(tb):
                xq_t = xs.tile([128, NCIN, 512], BF16, tag="xs", name="xq_t")
                for ci in range(NCIN):
                    nc.default_dma_engine.dma_start(
                        out=xq_t[:, ci, :],
                        in_=xq_r[:, ci, tb * 512:(tb + 1) * 512])
                for co in range(NCOUT):
                    ps = pp.tile([128, 512], F32, tag="proj", name="psq")
                    for ci in range(NCIN):
                        nc.tensor.matmul(
                            ps, wq_sb[:, ci, co * 128:(co + 1) * 128],
                            xq_t[:, ci, :],
                            start=(ci == 0), stop=(ci == NCIN - 1))
                    nc.vector.tensor_scalar_add(
                        qt[co][:, tb * 512:(tb + 1) * 512], ps,
                        bq_sb[:, co:co + 1])

            # t-block 0 of K and Q, interleaved per chunk: scores (qb0, p0)
            # only need chunk co=0 of each, so exp starts ~12us in.
            k_proj_block(0, interleave_q=True)

            # Reserve priority space for the qb0 attention block: everything
            # emitted below (K t-blocks 1-3, V projection, Q t-blocks 1-3)
            # runs in qb0's PE slack, ordered K > V > Q by urgency.
            pri_mark = tc.cur_priority
            tc.cur_priority += 4000

            for tb in range(1, NTB):
                k_proj_block(tb)

            # ---------- V projection ----------
            for ci in range(NCIN):
                nc.default_dma_engine.dma_start(out=wv_sb[:, ci, :],
                                                in_=wv_r[:, ci, :])
            nc.default_dma_engine.dma_start(
                out=wo_sb, in_=wo.rearrange("(c p) g -> p c g", p=128))
            for tb in range(NTB):
                xv_t = xs.tile([128, NCIN, 512], BF16, tag="xv", name="xv_t")
                for ci in range(NCIN):
                    nc.default_dma_engine.dma_start(
                        out=xv_t[:, ci, :],
                        in_=xv_r[:, ci, tb * 512:(tb + 1) * 512])
                for sub in range(4):
                    tcix = tb * 4 + sub
                    ps = pp.tile([128, 512], F32, tag="proj", name="psv")
                    for ci in range(NCIN):
                        nc.tensor.matmul(
                            ps, xv_t[:, ci, sub * 128:(sub + 1) * 128],
                            wv_sb[:, ci, :],
                            start=(ci == 0), stop=(ci == NCIN - 1))
                    nc.vector.tensor_copy(out=v_aug[:, tcix, :, 0:64], in_=ps)

            for tb in range(1, NTB):
                q_proj_block(tb)

            def attention_qb(qb):
                qsl = slice(qb * 512, (qb + 1) * 512)
                ot_t = otp.tile([128, NCOUT, 512], BF16, tag="ot", name="ot_t")
                for p in range(NCOUT):
                    e01 = eb.tile([128, NKC, 2, 512], BF16, tag="e01",
                                  name="e01")
                    for kc in range(NKC):
                        psc = scp.tile([128, 2, 512], F32, tag="sc",
                                       name="psc")
                        ksl = slice(kc * 128, (kc + 1) * 128)
                        nc.tensor.matmul(
                            psc[:, 0, :], kt[p][0:64, ksl],
                            qt[p][0:64, qsl], start=True, stop=True)
                        nc.tensor.matmul(
                            psc[:, 1, :], kt[p][64:128, ksl],
                            qt[p][64:128, qsl], start=True, stop=True)
                        nc.scalar.activation(e01[:, kc, :, :], psc, AF.Exp)
                    # P @ V with ones-augmented V: row 64 = denominator
                    pv0 = pvp.tile([128, 512], F32, tag="pv", name="pv0")
                    pv1 = pvp.tile([128, 512], F32, tag="pv", name="pv1")
                    for kc in range(NKC):
                        nc.tensor.matmul(
                            pv0[0:65, :], v_aug[:, kc, 2 * p, :],
                            e01[:, kc, 0, :],
                            start=(kc == 0), stop=(kc == NKC - 1))
                        nc.tensor.matmul(
                            pv1[0:65, :], v_aug[:, kc, 2 * p + 1, :],
                            e01[:, kc, 1, :],
                            start=(kc == 0), stop=(kc == NKC - 1))
                    d_sb = dv.tile([1, 2, 512], F32, tag="dsb", name="d_sb")
                    nc.vector.tensor_copy(out=d_sb[0:1, 0, :],
                                          in_=pv0[64:65, :])
                    nc.vector.tensor_copy(out=d_sb[0:1, 1, :],
                                          in_=pv1[64:65, :])
                    rec = dv.tile([1, 2, 512], F32, tag="rec", name="rec")
                    nc.vector.reciprocal_approx_fast(rec[0:1, :, :],
                                                     d_sb[0:1, :, :])
                    dbc_lo = dv.tile([64, 512], F32, tag="b0", name="dbc_lo")
                    dbc_hi = dv.tile([64, 512], F32, tag="b1", name="dbc_hi")
                    nc.gpsimd.partition_broadcast(dbc_lo[:, :], rec[0:1, 0, :],
                                                  channels=64)
                    nc.gpsimd.partition_broadcast(dbc_hi[:, :], rec[0:1, 1, :],
                                                  channels=64)
                    nc.vector.tensor_mul(ot_t[0:64, p, :], pv0[0:64, :],
                                         dbc_lo[:, :])
                    nc.vector.tensor_mul(ot_t[64:128, p, :], pv1[0:64, :],
                                         dbc_hi[:, :])
                # out-projection for this query block
                for tcx in range(4):
                    for n in range(2):
                        pj = pvp.tile([128, 512], F32, tag="pv", name="pj")
                        for p in range(NCOUT):
                            nc.tensor.matmul(
                                pj, ot_t[:, p, tcx * 128:(tcx + 1) * 128],
                                wo_sb[:, p, n * 512:(n + 1) * 512],
                                start=(p == 0), stop=(p == NCOUT - 1))
                        oj = dv.tile([128, 512], BF16, tag="oj", bufs=2,
                                     name="oj")
                        nc.vector.tensor_copy(out=oj, in_=pj)
                        r0 = qb * 512 + tcx * 128
                        nc.default_dma_engine.dma_start(
                            out=out[r0:r0 + 128, n * 512:(n + 1) * 512],
                            in_=oj)

            # qb0 attention goes into the reserved priority slot right after
            # the (K0, Q0) projections; later qbs follow in program order.
            with tc.high_priority(offset=tc.cur_priority - pri_mark):
                attention_qb(0)
            for qb in range(1, NQB):
                attention_qb(qb)
    nc.finalize()
    return nc


_CACHE = {}


def _get_runner():
    """Compile once per process; return f(in_maps) -> list of out dicts."""
    if "runner" in _CACHE:
        return _CACHE["runner"]
    import jax
    from jax.sharding import Mesh, PartitionSpec
    from jax.experimental.shard_map import shard_map
    from concourse import bass2jax

    nc = build_nc()
    bass2jax.install_neuronx_cc_hook()
    in_names, out_names, out_avals, zero_shapes = [], [], [], []
    for alloc in nc.m.functions[0].allocations:
        if not isinstance(alloc, mybir.MemoryLocationSet):
            continue
        name = alloc.memorylocations[0].name
        if alloc.kind == "ExternalInput":
            if name != "partition_id":
                in_names.append(name)
        elif alloc.kind == "ExternalOutput":
            out_names.append(name)
            shape = tuple(alloc.tensor_shape)
            dtype = mybir.dt.np(alloc.dtype)
            out_avals.append(jax.core.ShapedArray(shape, dtype))
            zero_shapes.append((shape, dtype))
    n_params = len(in_names)
    all_names = tuple(in_names + out_names)
    donate = tuple(range(n_params, n_params + len(out_names)))
    has_pid = nc.partition_id_tensor is not None

    def _body(*args):
        operands = list(args)
        names = all_names
        if has_pid:
            operands.append(bass2jax.partition_id_tensor())
            names = all_names + ("partition_id",)
        outs = bass2jax._bass_exec_p.bind(
            *operands, out_avals=tuple(out_avals), in_names=names,
            out_names=tuple(out_names), lowering_input_output_aliases=(),
            sim_require_finite=False, sim_require_nnan=False, nc=nc)
        return tuple(outs)

    devices = jax.devices()[:8]
    mesh = Mesh(np.asarray(devices), ("core",))
    specs = (PartitionSpec("core"),) * (n_params + len(out_names))
    f = jax.jit(shard_map(_body, mesh=mesh, in_specs=specs,
                          out_specs=(PartitionSpec("core"),) * len(out_names),
                          check_rep=False),
                donate_argnums=donate, keep_unused=True)

    def run(in_maps):
        concat_in = [np.concatenate([m[n] for m in in_maps], axis=0)
                     for n in in_names]
        concat_zeros = [np.zeros((8 * s[0], *s[1:]), d) for s, d in zero_shapes]
        outs = f(*concat_in, *concat_zeros)
        res = []
        for c in range(8):
            res.append({name: np.asarray(outs[i]).reshape(8, *out_avals[i].shape)[c]
                        for i, name in enumerate(out_names)})
        return res

    _CACHE["runner"] = run
    _CACHE["nc"] = nc
    return run


def make_in_maps(k, q, v, Wk, bk, Wq, bq, Wv, bv, Wo, bo):
    in_maps = []
    for c in range(8):
        b, g = divmod(c, 2)
        gs, ge = g * G, (g + 1) * G
        bqs = (bq[gs:ge] * SCALE).reshape(NCOUT, 128).T
        in_maps.append({
            "xq": np.ascontiguousarray(q[b].T).astype(ml_dtypes.bfloat16),
            "xk": np.ascontiguousarray(k[b].T).astype(ml_dtypes.bfloat16),
            "xv": np.ascontiguousarray(v[b].T).astype(ml_dtypes.bfloat16),
            "wq": np.ascontiguousarray(Wq[gs:ge, :].T * SCALE).astype(
                ml_dtypes.bfloat16),
            "wk": np.ascontiguousarray(Wk[gs:ge, :].T).astype(
                ml_dtypes.bfloat16),
            "wv": np.ascontiguousarray(Wv[gs:ge, :].T).astype(
                ml_dtypes.bfloat16),
            "wo": np.ascontiguousarray(Wo[:, gs:ge].T).astype(
                ml_dtypes.bfloat16),
            "bq": np.ascontiguousarray(bqs, dtype=np.float32),
        })
    return in_maps


def kernel(k, q, v, Wk, bk, Wq, bq, Wv, bv, Wo, bo):
    k = np.asarray(k, dtype=np.float32)
    q = np.asarray(q, dtype=np.float32)
    v = np.asarray(v, dtype=np.float32)
    Wk, bk = np.asarray(Wk, np.float32), np.asarray(bk, np.float32)
    Wq, bq = np.asarray(Wq, np.float32), np.asarray(bq, np.float32)
    Wv, bv = np.asarray(Wv, np.float32), np.asarray(bv, np.float32)
    Wo, bo = np.asarray(Wo, np.float32), np.asarray(bo, np.float32)

    in_maps = make_in_maps(k, q, v, Wk, bk, Wq, bq, Wv, bv, Wo, bo)
    run = _get_runner()
    res = run(in_maps)
    host_bias = (bo + Wo @ bv).astype(np.float32)
    out = np.empty((B, T, C), np.float32)
    for b in range(B):
        out[b] = (res[2 * b]["out"].astype(np.float32)
                  + res[2 * b + 1]["out"].astype(np.float32)
                  + host_bias[None, :])
    return out
